# revision 45
# baseline (speedup 1.0000x reference)
"""Distributed multi-head causal attention with RoPE on 8 TRN2 NeuronCores.

Sharding: batch (2) x head-groups (4 heads each) -> 8 cores.
  core c: batch b = c // 4, head group g = c % 4 (global heads 4g..4g+3).

Per-core kernel (all matmuls bf16, fp32 accumulate):
  1. QKV projections in transposed layout: QT/KT[d, seq] (head dim on
     partitions), V[seq, dv] natural.  RoPE pair-interleave is folded into a
     host-side permutation of wq/wk rows (evens-first), so on-device RoPE is
     3 elementwise ops + a half-swap copy.  The 1/sqrt(hd) scale is folded
     into wq on the host.
  2. Scores computed transposed: ST[k, q] = KT_blk.T @ QT (so softmax'd
     probabilities come out in the exact layout PV needs as its moving
     operand).  exp on ACT (no max subtraction -- scores are O(1) for this
     problem), fully-masked blocks skipped structurally, partial blocks
     masked by a 0/1 pattern multiply.  Column sums via an all-ones [128,128]
     matmul (gives the sum replicated across partitions, so the reciprocal
     runs full-width); normalization is applied to the PV output.
  3. Per-head AllGather (bf16) of normalized attnT across the 4 cores of the
     batch group, overlapped with attention of later heads; gathered heads
     are staged back into SBUF as they arrive.
  4. Output projection column-sharded: each core computes its 512 output
     columns from the gathered attnT; host concatenates.
"""

import functools
import math

import numpy as np
import ml_dtypes

BSZ, SEQ, DIM, NH, HD = 2, 2048, 2048, 16, 128
NCORES = 8
GSIZE = 4            # cores per batch group
HPC = NH // GSIZE    # heads per core = 4
DLOC = HPC * HD      # local head dims = 512
QC = 512             # q-chunk (matmul moving free dim)
NQC = SEQ // QC      # 4
KT = 128             # k-tile
NKT = SEQ // KT      # 16
IC = 128             # contraction tile
NIC = DIM // IC      # 16
BF16 = ml_dtypes.bfloat16
NEG_BIG = -30000.0


def _build_and_compile(block_plan_key, n_pat):
    """Build + compile the SPMD bass graph.  block_plan_key is a tuple over
    q-chunks of tuples of (kt, pat_idx or -1)."""
    import concourse.bass as bass
    import concourse.tile as tile
    from concourse import bacc, mybir
    from contextlib import ExitStack

    f32 = mybir.dt.float32
    bf16 = mybir.dt.bfloat16
    ts = bass.ts

    # entries: (kt, pat_idx or None).  NOTE: matmul PSUM writes must start at
    # the bank base -- column-offset writes are a fatal HW error -- so all
    # score/PV/colsum matmuls run full q-chunk width.
    block_plan = [[(kt, (None if p < 0 else p)) for kt, p in qcp]
                  for qcp in block_plan_key]

    nc = bacc.Bacc("TRN2", target_bir_lowering=False, debug=False,
                   num_devices=NCORES)

    # all inputs host-prepped into partition-major layouts so every DMA is
    # a contiguous (or large-run) transfer
    xT_d = nc.dram_tensor("xT", [128, NQC, NIC, QC], bf16,
                          kind="ExternalInput").ap()
    wq0_d = nc.dram_tensor("wq0T", [128, NIC, HD], bf16,
                           kind="ExternalInput").ap()
    wk0_d = nc.dram_tensor("wk0T", [128, NIC, HD], bf16,
                           kind="ExternalInput").ap()
    wqR_d = nc.dram_tensor("wqRT", [128, NIC, DLOC - HD], bf16,
                           kind="ExternalInput").ap()
    wkR_d = nc.dram_tensor("wkRT", [128, NIC, DLOC - HD], bf16,
                           kind="ExternalInput").ap()
    wvT_d = nc.dram_tensor("wvT", [128, NIC, DLOC], bf16,
                           kind="ExternalInput").ap()
    woT_d = nc.dram_tensor("woT", [128, NIC, DLOC], bf16,
                           kind="ExternalInput").ap()
    cos2_d = nc.dram_tensor("cos2", [HD, SEQ], bf16, kind="ExternalInput").ap()
    sinpm_d = nc.dram_tensor("sinpm", [HD, SEQ], bf16,
                             kind="ExternalInput").ap()
    pat_d = nc.dram_tensor("pat", [128, max(n_pat, 1), QC], bf16,
                           kind="ExternalInput").ap()
    out_d = nc.dram_tensor("out", [SEQ, DLOC], f32, kind="ExternalOutput").ap()

    groups = [[0, 1, 2, 3], [4, 5, 6, 7]]

    with tile.TileContext(nc) as tc, ExitStack() as top:
        persist = top.enter_context(tc.tile_pool(name="persist", bufs=1))
        dram = top.enter_context(
            tc.tile_pool(name="dram", bufs=2 * HPC, space="DRAM"))

        qt_sb = persist.tile([128, HPC, SEQ], bf16, name="qt_sb")
        kt_sb = persist.tile([128, HPC, SEQ], bf16, name="kt_sb")
        v_sb = persist.tile([128, NKT, DLOC], bf16, name="v_sb")
        at_sb = persist.tile([128, HPC, SEQ], bf16, name="at_sb")
        ones_sb = persist.tile([128, 128], bf16, name="ones_sb")
        pat_sb = persist.tile([128, max(n_pat, 1), QC], bf16, name="pat_sb")

        nc.vector.memset(ones_sb[:], 1.0)

        # attention-phase pools (opened below phase A's pools so phase A can
        # release in stack order while these persist)
        ptpool = top.enter_context(tc.tile_pool(name="ptpool", bufs=8))
        rbpool = top.enter_context(tc.tile_pool(name="rbpool", bufs=2))

        # ---------------- Phase A: QKV projections + RoPE ----------------
        # Split in two passes: pass 1 produces head 0's Q/K plus all of V, so
        # head 0's attention -- and with it the serialized AllGather chain --
        # can start ~100us earlier.  Pass 2 produces heads 1..3 and overlaps
        # head 0's attention epilogue.
        pa = top.enter_context(ExitStack())
        wpool = pa.enter_context(tc.tile_pool(name="wpool", bufs=1))
        xpool = pa.enter_context(tc.tile_pool(name="xpool", bufs=16))
        rpool = pa.enter_context(tc.tile_pool(name="rope", bufs=4))
        cpool = pa.enter_context(tc.tile_pool(name="cospool", bufs=1))

        XG = 2                      # x chunks per DMA group
        wq0_sb = wpool.tile([128, NIC, HD], bf16, name="wq0_sb")
        wk0_sb = wpool.tile([128, NIC, HD], bf16, name="wk0_sb")
        wqR_sb = wpool.tile([128, NIC, DLOC - HD], bf16, name="wqR_sb")
        wkR_sb = wpool.tile([128, NIC, DLOC - HD], bf16, name="wkR_sb")
        wv_sb = wpool.tile([128, NIC, DLOC], bf16, name="wv_sb")
        cos2_sb = cpool.tile([HD, SEQ], bf16, name="cos2_sb")
        sinpm_sb = cpool.tile([HD, SEQ], bf16, name="sinpm_sb")

        def load_x(qc):
            """x(qc) as NIC//XG group tiles of [128, XG, QC]."""
            tiles = []
            for g in range(NIC // XG):
                t = xpool.tile([128, XG, QC], bf16, name="x_sb", tag="x_sb")
                eng = nc.sync if g % 2 == 0 else nc.gpsimd
                eng.dma_start(
                    out=t[:],
                    in_=xT_d[:, qc, bass.ds(g * XG, XG), :])
                tiles.append(t)
            return tiles

        def x_at(xs, ic):
            return xs[ic // XG][:, ic % XG, :]

        # Startup DMA priority: consts + head-0 weight slices + x(qc0) + wv
        # first (pass 1's working set, spread over 3 queues); the rest of
        # wq/wk can land any time before pass 2.
        for qq in range(4):
            sl = bass.ds(qq * (NIC // 4), NIC // 4)
            nc.scalar.dma_start(out=wq0_sb[:, sl, :], in_=wq0_d[:, sl, :])
            nc.scalar.dma_start(out=wk0_sb[:, sl, :], in_=wk0_d[:, sl, :])
        nc.scalar.dma_start(out=cos2_sb[:], in_=cos2_d[:, :])
        nc.scalar.dma_start(out=sinpm_sb[:], in_=sinpm_d[:, :])
        x_sb = {}
        x_sb[0] = []
        for g in range(NIC // XG):
            t = xpool.tile([128, XG, QC], bf16, name="x_sb", tag="x_sb")
            eng = nc.sync if g % 2 == 0 else nc.gpsimd
            eng.dma_start(out=t[:], in_=xT_d[:, 0, bass.ds(g * XG, XG), :])
            eng.dma_start(out=wv_sb[:, bass.ds(g * XG, XG), :],
                          in_=wvT_d[:, bass.ds(g * XG, XG), :])
            x_sb[0].append(t)
        nc.scalar.dma_start(out=pat_sb[:], in_=pat_d[:, :, :])
        for qq in range(8):
            sl = bass.ds(qq * (NIC // 8), NIC // 8)
            nc.scalar.dma_start(out=wqR_sb[:, sl, :], in_=wqR_d[:, sl, :])
            nc.scalar.dma_start(out=wkR_sb[:, sl, :], in_=wkR_d[:, sl, :])

        def rope_evict(acc, dst_slice, qc):
            sw = rpool.tile([128, QC], f32, name="sw", tag="sw")
            m1 = rpool.tile([128, QC], f32, name="m1", tag="m1")
            nc.vector.tensor_copy(sw[0:64, :], acc[64:128, :])
            nc.vector.tensor_copy(sw[64:128, :], acc[0:64, :])
            nc.vector.tensor_mul(m1[:], acc[:], cos2_sb[:, ts(qc, QC)])
            nc.vector.tensor_mul(sw[:], sw[:], sinpm_sb[:, ts(qc, QC)])
            nc.vector.tensor_add(dst_slice, m1[:], sw[:])

        with ExitStack() as pa1:
            ps_a = pa1.enter_context(
                tc.tile_pool(name="ps_a1", bufs=3, space="PSUM"))
            for qc in range(NQC):
                if qc + 1 < NQC:
                    x_sb[qc + 1] = load_x(qc + 1)
                xs = x_sb[qc]
                # head 0 Q and K
                for w_sb, dst in ((wq0_sb, qt_sb), (wk0_sb, kt_sb)):
                    acc = ps_a.tile([128, QC], f32, name="acc", tag="ps_a")
                    for ic in range(NIC):
                        nc.tensor.matmul(acc[:], w_sb[:, ic, :],
                                         x_at(xs, ic),
                                         start=(ic == 0), stop=(ic == NIC - 1))
                    rope_evict(acc, dst[:, 0, ts(qc, QC)], qc)
                # V pass (natural layout, all local heads)
                for sl in range(QC // 128):
                    s = qc * (QC // 128) + sl
                    acc = ps_a.tile([128, DLOC], f32, name="acc", tag="ps_a")
                    for ic in range(NIC):
                        nc.tensor.matmul(
                            acc[:], x_at(xs, ic)[:, ts(sl, 128)],
                            wv_sb[:, ic, :],
                            start=(ic == 0), stop=(ic == NIC - 1))
                    nc.vector.tensor_copy(v_sb[:, s, :], acc[:])

        # ---------------- Phase B: attention ----------------
        ag_outs = {}   # (head, half) -> gathered DRAM buffer [512, SEQ//2]

        EB = 2   # k-tiles per exp batch

        def attn_head(h, ps_st, ps_pv, ps_on):
            for qc in range(NQC):
                kts = block_plan[qc]
                nkt = len(kts)
                pv = ps_pv.tile([128, QC], f32, name="pv", tag="pv")
                csum = ps_on.tile([128, QC], f32, name="csum", tag="csum")
                # batch k-tiles so exp runs on [128, EB*QC] blocks
                for pi in range(0, nkt, EB):
                    pair = kts[pi:pi + EB]
                    st = ps_st.tile([128, EB, QC], f32, name="st", tag="st")
                    for j, (kt, _) in enumerate(pair):
                        nc.tensor.matmul(
                            st[:, j, :],
                            kt_sb[:, h, ts(kt, KT)],
                            qt_sb[:, h, ts(qc, QC)],
                            start=True, stop=True)
                    pt = ptpool.tile([128, EB, QC], bf16, name="pt", tag="pt")
                    nc.scalar.activation(
                        pt[:, 0:len(pair), :], st[:, 0:len(pair), :],
                        mybir.ActivationFunctionType.Exp)
                    for j, (kt, pidx) in enumerate(pair):
                        if pidx is not None:
                            nc.vector.tensor_mul(
                                pt[:, j, :], pt[:, j, :],
                                pat_sb[:, pidx, :])
                        i = pi + j
                        nc.tensor.matmul(
                            csum[:], ones_sb[:],
                            pt[:, j, :],
                            start=(i == 0), stop=(i == nkt - 1))
                        nc.tensor.matmul(
                            pv[:], v_sb[:, kt, ts(h, HD)],
                            pt[:, j, :],
                            start=(i == 0), stop=(i == nkt - 1))
                # csum rows are all identical (ones matmul), so the
                # reciprocal runs full-width straight out of PSUM.
                rb = rbpool.tile([128, QC], f32, name="rb", tag="rb")
                nc.vector.reciprocal_approx_fast(rb[:], csum[:])
                nc.vector.tensor_mul(at_sb[:, h, ts(qc, QC)], pv[:], rb[:])

                # AllGather across the batch group once this head is done
                if qc == NQC - 1:
                    ag_in = dram.tile([128, SEQ], bf16, name="ag_in",
                                      tag="ag_in")
                    ag_out = dram.tile([GSIZE * 128, SEQ], bf16,
                                       name="ag_out", tag=f"ag_out{h}")
                    nc.sync.dma_start(out=ag_in[:], in_=at_sb[:, h, :])
                    nc.gpsimd.collective_compute(
                        "AllGather", mybir.AluOpType.bypass,
                        replica_groups=groups,
                        ins=[ag_in[:].opt()],
                        outs=[ag_out[:].opt()])
                    ag_outs[(h, 0)] = ag_out
                    ag_outs[(h, 1)] = ag_out

        # head 0 attention (kicks off the AllGather chain early)
        with ExitStack() as pb0:
            ps_st = pb0.enter_context(
                tc.tile_pool(name="ps_st0", bufs=2, space="PSUM"))
            ps_pv = pb0.enter_context(
                tc.tile_pool(name="ps_pv0", bufs=2, space="PSUM"))
            ps_on = pb0.enter_context(
                tc.tile_pool(name="ps_on0", bufs=2, space="PSUM"))
            attn_head(0, ps_st, ps_pv, ps_on)

        # pass 2: Q/K for heads 1..3
        with ExitStack() as pa2:
            ps_a = pa2.enter_context(
                tc.tile_pool(name="ps_a2", bufs=3, space="PSUM"))
            for qc in range(NQC):
                xs = x_sb[qc] = load_x(qc)
                for w_sb, dst in ((wqR_sb, qt_sb), (wkR_sb, kt_sb)):
                    for h in range(1, HPC):
                        acc = ps_a.tile([128, QC], f32, name="acc2",
                                        tag="ps_a2")
                        for ic in range(NIC):
                            nc.tensor.matmul(
                                acc[:], w_sb[:, ic, ts(h - 1, HD)],
                                x_at(xs, ic),
                                start=(ic == 0), stop=(ic == NIC - 1))
                        rope_evict(acc, dst[:, h, ts(qc, QC)], qc)
        pa.close()  # release x/weights/rope pools

        # heads 1..3 attention
        with ExitStack() as pb:
            ps_st = pb.enter_context(
                tc.tile_pool(name="ps_st", bufs=2, space="PSUM"))
            ps_pv = pb.enter_context(
                tc.tile_pool(name="ps_pv", bufs=2, space="PSUM"))
            ps_on = pb.enter_context(
                tc.tile_pool(name="ps_on", bufs=2, space="PSUM"))
            for h in range(1, HPC):
                attn_head(h, ps_st, ps_pv, ps_on)

        # ---------------- Phase C: output projection ----------------
        # wo preload overlaps attention of heads 1..3
        wopool = top.enter_context(tc.tile_pool(name="wopool", bufs=1))
        wo_sb = wopool.tile([128, NIC, DLOC], bf16, name="wo_sb")
        for qq in range(8):
            sl = bass.ds(qq * (NIC // 8), NIC // 8)
            nc.sync.dma_start(out=wo_sb[:, sl, :], in_=woT_d[:, sl, :])
        # Two stages: heads 0..HPC-2 accumulate while the last head's
        # AllGather is still in flight; the last head's contribution is added
        # on top afterwards, so the AG tail hides behind ~192 matmuls.
        SH = NKT // 2   # s-tiles per AG half

        with ExitStack() as pc:
            popool = pc.enter_context(tc.tile_pool(name="popool", bufs=1))
            opool = pc.enter_context(tc.tile_pool(name="opool", bufs=3))
            ltpool = pc.enter_context(tc.tile_pool(name="ltpool", bufs=10))
            ps_c = pc.enter_context(
                tc.tile_pool(name="ps_c", bufs=2, space="PSUM"))
            ps_c2 = pc.enter_context(
                tc.tile_pool(name="ps_c2", bufs=2, space="PSUM"))

            po_sb = popool.tile([128, NKT, DLOC], f32, name="po_sb")

            lt_cache = {}

            def load_lt(h, s):
                """Gathered attnT [128, GSIZE, 128] for (local head h, s-tile).
                Loads two s-tiles per DMA (longer runs, half the issues)."""
                key = (h, s // 2)
                if key not in lt_cache:
                    lt = ltpool.tile([128, GSIZE, 256], bf16, name="lt",
                                     tag="lt")
                    src = ag_outs[(h, s // SH)].rearrange(
                        "(r p) s -> p r s", p=128)
                    eng = nc.sync if (h + s // 2) % 2 == 0 else nc.gpsimd
                    eng.dma_start(
                        out=lt[:],
                        in_=src[:, :, ts(s // 2, 256)])
                    lt_cache[key] = lt
                return lt_cache[key][:, :, ts(s % 2, 128)]

            for s in range(NKT):
                acc = ps_c.tile([128, DLOC], f32, name="acc_c")
                for h in range(HPC - 1):
                    lt = load_lt(h, s)
                    for r in range(GSIZE):
                        nc.tensor.matmul(
                            acc[:], lt[:, r, :],
                            wo_sb[:, GSIZE * r + h, :],
                            start=((h, r) == (0, 0)),
                            stop=((h, r) == (HPC - 2, GSIZE - 1)))
                nc.vector.tensor_copy(po_sb[:, s, :], acc[:])
            h = HPC - 1
            for s in range(NKT):
                acc2 = ps_c2.tile([128, DLOC], f32, name="acc2_c")
                lt = load_lt(h, s)
                for r in range(GSIZE):
                    nc.tensor.matmul(
                        acc2[:], lt[:, r, :],
                        wo_sb[:, GSIZE * r + h, :],
                        start=(r == 0), stop=(r == GSIZE - 1))
                ot = opool.tile([128, DLOC], f32, name="ot")
                nc.vector.tensor_add(ot[:], acc2[:], po_sb[:, s, :])
                nc.sync.dma_start(out=out_d[ts(s, 128), :], in_=ot[:])

    nc.compile()
    return nc


_CACHE = {}


def _get_compiled(block_plan_key, n_pat):
    key = (block_plan_key, n_pat)
    if key not in _CACHE:
        _CACHE[key] = _build_and_compile(block_plan_key, n_pat)
    return _CACHE[key]


def _plan_from_mask(mask):
    """Derive per-q-chunk k-tile lists + dedup'd 0/1 patterns from the mask.

    Plan entries are (kt, pat_idx or -1, q0): the leading q0 columns of the
    q-chunk are fully masked (skipped in the matmuls); a [KT, KT] 0/1 pattern
    multiplies columns [q0, q0+KT); everything beyond q0+KT must be all-kept.
    """
    keep = mask > -1e20
    if not np.all(mask[keep] == 0.0):
        raise NotImplementedError("only 0/-inf style masks supported")
    pats = []
    pat_index = {}
    plan = []
    for qc in range(NQC):
        qs = slice(qc * QC, (qc + 1) * QC)
        row = []
        for kt in range(NKT):
            ks = slice(kt * KT, (kt + 1) * KT)
            blk = keep[qs, ks]            # [QC, KT]
            if not blk.any():
                continue
            if blk.all():
                row.append((kt, -1))
                continue
            p = np.ascontiguousarray(blk.T).astype(np.float32)  # [KT, QC]
            kb = p.tobytes()
            if kb not in pat_index:
                pat_index[kb] = len(pats)
                pats.append(p)
            row.append((kt, pat_index[kb]))
        plan.append(tuple(row))
    return tuple(plan), pats


def _head_perm():
    """Row permutation per head: even dims first, then odd."""
    perm = []
    for h in range(NH):
        base = h * HD
        perm.extend(base + np.arange(0, HD, 2))
        perm.extend(base + np.arange(1, HD, 2))
    return np.array(perm)


def _pmajor(wT, lo=0, hi=None):
    """[DIM, D] (already transposed weight) -> [128, NIC, hi-lo] partition-
    major layout: out[p, c, d] = wT[c*128 + p, lo + d]."""
    hi = wT.shape[1] if hi is None else hi
    return np.ascontiguousarray(
        wT[:, lo:hi].reshape(NIC, 128, hi - lo).transpose(1, 0, 2)
    ).astype(BF16)


def _prep_in_maps(x, wq, wk, wv, wo, freqs_cos, freqs_sin, pats, n_pat):
    perm = _head_perm()
    wq_p = (wq / math.sqrt(HD))[perm]
    wk_p = wk[perm]

    cosT = np.ascontiguousarray(freqs_cos.T)        # [64, SEQ]
    sinT = np.ascontiguousarray(freqs_sin.T)
    cos2 = np.concatenate([cosT, cosT], axis=0).astype(BF16)   # [128, SEQ]
    sinpm = np.concatenate([-sinT, sinT], axis=0).astype(BF16)

    if n_pat:
        pat_np = np.stack(pats)                     # [n_pat, KT, QC]
    else:
        pat_np = np.zeros((1, KT, QC), dtype=np.float32)
    pat_h = np.ascontiguousarray(pat_np.transpose(1, 0, 2)).astype(BF16)

    # x[b].T -> [128, NQC, NIC, QC]: xh[p, qc, c, q] = xT[c*128+p, qc*QC+q]
    xh = []
    for b in range(BSZ):
        xT = x[b].T.reshape(NIC, 128, NQC, QC)
        xh.append(np.ascontiguousarray(
            xT.transpose(1, 2, 0, 3)).astype(BF16))

    in_maps = []
    for c in range(NCORES):
        b, g = c // GSIZE, c % GSIZE
        rows = slice(g * DLOC, (g + 1) * DLOC)
        wqT = wq_p[rows].T   # [DIM, DLOC]
        wkT = wk_p[rows].T
        in_maps.append({
            "xT": xh[b],
            "wq0T": _pmajor(wqT, 0, HD),
            "wqRT": _pmajor(wqT, HD, DLOC),
            "wk0T": _pmajor(wkT, 0, HD),
            "wkRT": _pmajor(wkT, HD, DLOC),
            "wvT": _pmajor(wv[rows].T),
            "woT": _pmajor(wo[rows].T),
            "cos2": cos2,
            "sinpm": sinpm,
            "pat": pat_h,
        })
    return in_maps


def kernel(x, wq, wk, wv, wo, freqs_cos, freqs_sin, mask):
    x = np.asarray(x, dtype=np.float32)
    wq = np.asarray(wq, dtype=np.float32)
    wk = np.asarray(wk, dtype=np.float32)
    wv = np.asarray(wv, dtype=np.float32)
    wo = np.asarray(wo, dtype=np.float32)
    freqs_cos = np.asarray(freqs_cos, dtype=np.float32)
    freqs_sin = np.asarray(freqs_sin, dtype=np.float32)
    mask = np.asarray(mask, dtype=np.float32)

    plan, pats = _plan_from_mask(mask)
    n_pat = len(pats)
    nc = _get_compiled(plan, n_pat)

    in_maps = _prep_in_maps(x, wq, wk, wv, wo, freqs_cos, freqs_sin,
                            pats, n_pat)

    from concourse.bass_utils import run_bass_kernel_spmd
    res = run_bass_kernel_spmd(nc, in_maps, core_ids=list(range(NCORES)))
    outs = res.results

    full = np.empty((BSZ, SEQ, DIM), dtype=np.float32)
    for c in range(NCORES):
        b, g = c // GSIZE, c % GSIZE
        full[b][:, g * DLOC:(g + 1) * DLOC] = outs[c]["out"]
    return full


# revision 46
# speedup vs baseline: 1.0526x; 1.0526x over previous
"""Distributed multi-head causal attention with RoPE on 8 TRN2 NeuronCores.

Sharding: batch (2) x head-groups (4 heads each) -> 8 cores.
  core c: batch b = c // 4, head group g = c % 4 (global heads 4g..4g+3).

Per-core kernel (all matmuls bf16, fp32 accumulate):
  1. QKV projections in transposed layout: QT/KT[d, seq] (head dim on
     partitions), V[seq, dv] natural.  RoPE pair-interleave is folded into a
     host-side permutation of wq/wk rows (evens-first), so on-device RoPE is
     3 elementwise ops + a half-swap copy.  The 1/sqrt(hd) scale is folded
     into wq on the host.
  2. Scores computed transposed: ST[k, q] = KT_blk.T @ QT (so softmax'd
     probabilities come out in the exact layout PV needs as its moving
     operand).  exp on ACT (no max subtraction -- scores are O(1) for this
     problem), fully-masked blocks skipped structurally, partial blocks
     masked by a 0/1 pattern multiply.  Column sums via an all-ones [128,128]
     matmul (gives the sum replicated across partitions, so the reciprocal
     runs full-width); normalization is applied to the PV output.
  3. Per-head AllGather (bf16) of normalized attnT across the 4 cores of the
     batch group, overlapped with attention of later heads; gathered heads
     are staged back into SBUF as they arrive.
  4. Output projection column-sharded: each core computes its 512 output
     columns from the gathered attnT; host concatenates.
"""

import functools
import math

import numpy as np
import ml_dtypes

BSZ, SEQ, DIM, NH, HD = 2, 2048, 2048, 16, 128
NCORES = 8
GSIZE = 4            # cores per batch group
HPC = NH // GSIZE    # heads per core = 4
DLOC = HPC * HD      # local head dims = 512
QC = 512             # q-chunk (matmul moving free dim)
NQC = SEQ // QC      # 4
KT = 128             # k-tile
NKT = SEQ // KT      # 16
IC = 128             # contraction tile
NIC = DIM // IC      # 16
BF16 = ml_dtypes.bfloat16
NEG_BIG = -30000.0


def _build_and_compile(block_plan_key, n_pat):
    """Build + compile the SPMD bass graph.  block_plan_key is a tuple over
    q-chunks of tuples of (kt, pat_idx or -1)."""
    import concourse.bass as bass
    import concourse.tile as tile
    from concourse import bacc, mybir
    from contextlib import ExitStack

    f32 = mybir.dt.float32
    bf16 = mybir.dt.bfloat16
    ts = bass.ts

    # entries: (kt, pat_idx or None).  NOTE: matmul PSUM writes must start at
    # the bank base -- column-offset writes are a fatal HW error -- so all
    # score/PV/colsum matmuls run full q-chunk width.
    block_plan = [[(kt, (None if p < 0 else p)) for kt, p in qcp]
                  for qcp in block_plan_key]

    nc = bacc.Bacc("TRN2", target_bir_lowering=False, debug=False,
                   num_devices=NCORES)

    # all inputs host-prepped into partition-major layouts so every DMA is
    # a contiguous (or large-run) transfer
    xT_d = nc.dram_tensor("xT", [128, NQC, NIC, QC], bf16,
                          kind="ExternalInput").ap()
    wq0_d = nc.dram_tensor("wq0T", [128, NIC, HD], bf16,
                           kind="ExternalInput").ap()
    wk0_d = nc.dram_tensor("wk0T", [128, NIC, HD], bf16,
                           kind="ExternalInput").ap()
    wqR_d = nc.dram_tensor("wqRT", [128, NIC, DLOC - HD], bf16,
                           kind="ExternalInput").ap()
    wkR_d = nc.dram_tensor("wkRT", [128, NIC, DLOC - HD], bf16,
                           kind="ExternalInput").ap()
    wvT_d = nc.dram_tensor("wvT", [128, NIC, DLOC], bf16,
                           kind="ExternalInput").ap()
    woT_d = nc.dram_tensor("woT", [128, NIC, DLOC], bf16,
                           kind="ExternalInput").ap()
    cos2_d = nc.dram_tensor("cos2", [HD, SEQ], bf16, kind="ExternalInput").ap()
    sinpm_d = nc.dram_tensor("sinpm", [HD, SEQ], bf16,
                             kind="ExternalInput").ap()
    pat_d = nc.dram_tensor("pat", [128, max(n_pat, 1), QC], bf16,
                           kind="ExternalInput").ap()
    out_d = nc.dram_tensor("out", [SEQ, DLOC], f32, kind="ExternalOutput").ap()

    groups = [[0, 1, 2, 3], [4, 5, 6, 7]]

    with tile.TileContext(nc) as tc, ExitStack() as top:
        persist = top.enter_context(tc.tile_pool(name="persist", bufs=1))
        dram = top.enter_context(
            tc.tile_pool(name="dram", bufs=2 * HPC, space="DRAM"))

        qt_sb = persist.tile([128, HPC, SEQ], bf16, name="qt_sb")
        kt_sb = persist.tile([128, HPC, SEQ], bf16, name="kt_sb")
        v_sb = persist.tile([128, NKT, DLOC], bf16, name="v_sb")
        at_sb = persist.tile([128, HPC, SEQ], bf16, name="at_sb")
        ones_sb = persist.tile([128, 128], bf16, name="ones_sb")
        pat_sb = persist.tile([128, max(n_pat, 1), QC], bf16, name="pat_sb")

        nc.vector.memset(ones_sb[:], 1.0)

        # attention-phase pools (opened below phase A's pools so phase A can
        # release in stack order while these persist)
        ptpool = top.enter_context(tc.tile_pool(name="ptpool", bufs=8))
        rbpool = top.enter_context(tc.tile_pool(name="rbpool", bufs=2))

        # ---------------- Phase A: QKV projections + RoPE ----------------
        # Split in two passes: pass 1 produces head 0's Q/K plus all of V, so
        # head 0's attention -- and with it the serialized AllGather chain --
        # can start ~100us earlier.  Pass 2 produces heads 1..3 and overlaps
        # head 0's attention epilogue.
        pa = top.enter_context(ExitStack())
        wpool = pa.enter_context(tc.tile_pool(name="wpool", bufs=1))
        xpool = pa.enter_context(tc.tile_pool(name="xpool", bufs=16))
        rpool = pa.enter_context(tc.tile_pool(name="rope", bufs=4))
        cpool = pa.enter_context(tc.tile_pool(name="cospool", bufs=1))

        XG = 2                      # x chunks per DMA group
        wq0_sb = wpool.tile([128, NIC, HD], bf16, name="wq0_sb")
        wk0_sb = wpool.tile([128, NIC, HD], bf16, name="wk0_sb")
        wqR_sb = wpool.tile([128, NIC, DLOC - HD], bf16, name="wqR_sb")
        wkR_sb = wpool.tile([128, NIC, DLOC - HD], bf16, name="wkR_sb")
        wv_sb = wpool.tile([128, NIC, DLOC], bf16, name="wv_sb")
        cos2_sb = cpool.tile([HD, SEQ], bf16, name="cos2_sb")
        sinpm_sb = cpool.tile([HD, SEQ], bf16, name="sinpm_sb")

        def load_x(qc):
            """x(qc) as NIC//XG group tiles of [128, XG, QC]."""
            tiles = []
            for g in range(NIC // XG):
                t = xpool.tile([128, XG, QC], bf16, name="x_sb", tag="x_sb")
                eng = nc.sync if g % 2 == 0 else nc.gpsimd
                eng.dma_start(
                    out=t[:],
                    in_=xT_d[:, qc, bass.ds(g * XG, XG), :])
                tiles.append(t)
            return tiles

        def x_at(xs, ic):
            return xs[ic // XG][:, ic % XG, :]

        # Startup DMA priority: consts + head-0 weight slices + x(qc0) + wv
        # first (pass 1's working set, spread over 3 queues); the rest of
        # wq/wk can land any time before pass 2.
        for qq in range(4):
            sl = bass.ds(qq * (NIC // 4), NIC // 4)
            nc.scalar.dma_start(out=wq0_sb[:, sl, :], in_=wq0_d[:, sl, :])
            nc.scalar.dma_start(out=wk0_sb[:, sl, :], in_=wk0_d[:, sl, :])
        nc.scalar.dma_start(out=cos2_sb[:], in_=cos2_d[:, :])
        nc.scalar.dma_start(out=sinpm_sb[:], in_=sinpm_d[:, :])
        x_sb = {}
        x_sb[0] = []
        for g in range(NIC // XG):
            t = xpool.tile([128, XG, QC], bf16, name="x_sb", tag="x_sb")
            eng = nc.sync if g % 2 == 0 else nc.gpsimd
            eng.dma_start(out=t[:], in_=xT_d[:, 0, bass.ds(g * XG, XG), :])
            eng.dma_start(out=wv_sb[:, bass.ds(g * XG, XG), :],
                          in_=wvT_d[:, bass.ds(g * XG, XG), :])
            x_sb[0].append(t)
        nc.scalar.dma_start(out=pat_sb[:], in_=pat_d[:, :, :])
        for qq in range(8):
            sl = bass.ds(qq * (NIC // 8), NIC // 8)
            nc.scalar.dma_start(out=wqR_sb[:, sl, :], in_=wqR_d[:, sl, :])
            nc.scalar.dma_start(out=wkR_sb[:, sl, :], in_=wkR_d[:, sl, :])

        def rope_evict(acc, dst_slice, qc):
            sw = rpool.tile([128, QC], f32, name="sw", tag="sw")
            m1 = rpool.tile([128, QC], f32, name="m1", tag="m1")
            nc.vector.tensor_copy(sw[0:64, :], acc[64:128, :])
            nc.vector.tensor_copy(sw[64:128, :], acc[0:64, :])
            nc.vector.tensor_mul(m1[:], acc[:], cos2_sb[:, ts(qc, QC)])
            nc.vector.tensor_mul(sw[:], sw[:], sinpm_sb[:, ts(qc, QC)])
            nc.vector.tensor_add(dst_slice, m1[:], sw[:])

        with ExitStack() as pa1:
            ps_a = pa1.enter_context(
                tc.tile_pool(name="ps_a1", bufs=3, space="PSUM"))
            for qc in range(NQC):
                if qc + 1 < NQC:
                    x_sb[qc + 1] = load_x(qc + 1)
                xs = x_sb[qc]
                # head 0 Q and K
                for w_sb, dst in ((wq0_sb, qt_sb), (wk0_sb, kt_sb)):
                    acc = ps_a.tile([128, QC], f32, name="acc", tag="ps_a")
                    for ic in range(NIC):
                        nc.tensor.matmul(acc[:], w_sb[:, ic, :],
                                         x_at(xs, ic),
                                         start=(ic == 0), stop=(ic == NIC - 1))
                    rope_evict(acc, dst[:, 0, ts(qc, QC)], qc)
                # V pass (natural layout, all local heads)
                for sl in range(QC // 128):
                    s = qc * (QC // 128) + sl
                    acc = ps_a.tile([128, DLOC], f32, name="acc", tag="ps_a")
                    for ic in range(NIC):
                        nc.tensor.matmul(
                            acc[:], x_at(xs, ic)[:, ts(sl, 128)],
                            wv_sb[:, ic, :],
                            start=(ic == 0), stop=(ic == NIC - 1))
                    nc.vector.tensor_copy(v_sb[:, s, :], acc[:])

        # ---------------- Phase B: attention ----------------
        ag_outs = {}   # (head, half) -> gathered DRAM buffer [512, SEQ//2]

        EB = 2   # k-tiles per exp batch

        def attn_head(h, ps_st, ps_pv, ps_on):
            for qc in range(NQC):
                kts = block_plan[qc]
                nkt = len(kts)
                pv = ps_pv.tile([128, QC], f32, name="pv", tag="pv")
                csum = ps_on.tile([128, QC], f32, name="csum", tag="csum")
                # batch k-tiles so exp runs on [128, EB*QC] blocks
                for pi in range(0, nkt, EB):
                    pair = kts[pi:pi + EB]
                    st = ps_st.tile([128, EB, QC], f32, name="st", tag="st")
                    for j, (kt, _) in enumerate(pair):
                        nc.tensor.matmul(
                            st[:, j, :],
                            kt_sb[:, h, ts(kt, KT)],
                            qt_sb[:, h, ts(qc, QC)],
                            start=True, stop=True)
                    pt = ptpool.tile([128, EB, QC], bf16, name="pt", tag="pt")
                    nc.scalar.activation(
                        pt[:, 0:len(pair), :], st[:, 0:len(pair), :],
                        mybir.ActivationFunctionType.Exp)
                    for j, (kt, pidx) in enumerate(pair):
                        if pidx is not None:
                            nc.vector.tensor_mul(
                                pt[:, j, :], pt[:, j, :],
                                pat_sb[:, pidx, :])
                        i = pi + j
                        nc.tensor.matmul(
                            csum[:], ones_sb[:],
                            pt[:, j, :],
                            start=(i == 0), stop=(i == nkt - 1))
                        nc.tensor.matmul(
                            pv[:], v_sb[:, kt, ts(h, HD)],
                            pt[:, j, :],
                            start=(i == 0), stop=(i == nkt - 1))
                # csum rows are all identical (ones matmul), so the
                # reciprocal runs full-width straight out of PSUM.
                rb = rbpool.tile([128, QC], f32, name="rb", tag="rb")
                nc.vector.reciprocal_approx_fast(rb[:], csum[:])
                nc.vector.tensor_mul(at_sb[:, h, ts(qc, QC)], pv[:], rb[:])

                # half-sequence AllGather across the batch group: fire as
                # soon as the first/second half of this head is done.
                if qc % 2 == 1:
                    half = qc // 2
                    hs = half * (SEQ // 2)
                    ag_in = dram.tile([128, SEQ // 2], bf16, name="ag_in",
                                      tag="ag_in")
                    ag_out = dram.tile([GSIZE * 128, SEQ // 2], bf16,
                                       name="ag_out", tag=f"ag_out{h}_{half}")
                    nc.sync.dma_start(
                        out=ag_in[:],
                        in_=at_sb[:, h, bass.ds(hs, SEQ // 2)])
                    nc.gpsimd.collective_compute(
                        "AllGather", mybir.AluOpType.bypass,
                        replica_groups=groups,
                        ins=[ag_in[:].opt()],
                        outs=[ag_out[:].opt()])
                    ag_outs[(h, half)] = ag_out

        # head 0 attention (kicks off the AllGather chain early)
        with ExitStack() as pb0:
            ps_st = pb0.enter_context(
                tc.tile_pool(name="ps_st0", bufs=2, space="PSUM"))
            ps_pv = pb0.enter_context(
                tc.tile_pool(name="ps_pv0", bufs=2, space="PSUM"))
            ps_on = pb0.enter_context(
                tc.tile_pool(name="ps_on0", bufs=2, space="PSUM"))
            attn_head(0, ps_st, ps_pv, ps_on)

        # pass 2: Q/K for heads 1..3
        with ExitStack() as pa2:
            ps_a = pa2.enter_context(
                tc.tile_pool(name="ps_a2", bufs=3, space="PSUM"))
            for qc in range(NQC):
                xs = x_sb[qc] = load_x(qc)
                for w_sb, dst in ((wqR_sb, qt_sb), (wkR_sb, kt_sb)):
                    for h in range(1, HPC):
                        acc = ps_a.tile([128, QC], f32, name="acc2",
                                        tag="ps_a2")
                        for ic in range(NIC):
                            nc.tensor.matmul(
                                acc[:], w_sb[:, ic, ts(h - 1, HD)],
                                x_at(xs, ic),
                                start=(ic == 0), stop=(ic == NIC - 1))
                        rope_evict(acc, dst[:, h, ts(qc, QC)], qc)
        pa.close()  # release x/weights/rope pools

        # heads 1..3 attention
        with ExitStack() as pb:
            ps_st = pb.enter_context(
                tc.tile_pool(name="ps_st", bufs=2, space="PSUM"))
            ps_pv = pb.enter_context(
                tc.tile_pool(name="ps_pv", bufs=2, space="PSUM"))
            ps_on = pb.enter_context(
                tc.tile_pool(name="ps_on", bufs=2, space="PSUM"))
            for h in range(1, HPC):
                attn_head(h, ps_st, ps_pv, ps_on)

        # ---------------- Phase C: output projection ----------------
        # wo preload overlaps attention of heads 1..3
        wopool = top.enter_context(tc.tile_pool(name="wopool", bufs=1))
        wo_sb = wopool.tile([128, NIC, DLOC], bf16, name="wo_sb")
        for qq in range(8):
            sl = bass.ds(qq * (NIC // 8), NIC // 8)
            nc.sync.dma_start(out=wo_sb[:, sl, :], in_=woT_d[:, sl, :])
        # Two stages: heads 0..HPC-2 accumulate while the last head's
        # AllGather is still in flight; the last head's contribution is added
        # on top afterwards, so the AG tail hides behind ~192 matmuls.
        SH = NKT // 2   # s-tiles per AG half

        with ExitStack() as pc:
            popool = pc.enter_context(tc.tile_pool(name="popool", bufs=1))
            opool = pc.enter_context(tc.tile_pool(name="opool", bufs=3))
            ltpool = pc.enter_context(tc.tile_pool(name="ltpool", bufs=10))
            ps_c = pc.enter_context(
                tc.tile_pool(name="ps_c", bufs=2, space="PSUM"))
            ps_c2 = pc.enter_context(
                tc.tile_pool(name="ps_c2", bufs=2, space="PSUM"))

            po_sb = popool.tile([128, NKT, DLOC], f32, name="po_sb")

            lt_cache = {}

            def load_lt(h, s):
                """Gathered attnT [128, GSIZE, 128] for (local head h, s-tile).
                Loads two s-tiles per DMA (longer runs, half the issues)."""
                key = (h, s // 2)
                if key not in lt_cache:
                    lt = ltpool.tile([128, GSIZE, 256], bf16, name="lt",
                                     tag="lt")
                    src = ag_outs[(h, s // SH)].rearrange(
                        "(r p) s -> p r s", p=128)
                    eng = nc.sync if (h + s // 2) % 2 == 0 else nc.gpsimd
                    eng.dma_start(
                        out=lt[:],
                        in_=src[:, :, ts((s % SH) // 2, 256)])
                    lt_cache[key] = lt
                return lt_cache[key][:, :, ts(s % 2, 128)]

            for s in range(NKT):
                acc = ps_c.tile([128, DLOC], f32, name="acc_c")
                for h in range(HPC - 1):
                    lt = load_lt(h, s)
                    for r in range(GSIZE):
                        nc.tensor.matmul(
                            acc[:], lt[:, r, :],
                            wo_sb[:, GSIZE * r + h, :],
                            start=((h, r) == (0, 0)),
                            stop=((h, r) == (HPC - 2, GSIZE - 1)))
                nc.vector.tensor_copy(po_sb[:, s, :], acc[:])
            h = HPC - 1
            for s in range(NKT):
                acc2 = ps_c2.tile([128, DLOC], f32, name="acc2_c")
                lt = load_lt(h, s)
                for r in range(GSIZE):
                    nc.tensor.matmul(
                        acc2[:], lt[:, r, :],
                        wo_sb[:, GSIZE * r + h, :],
                        start=(r == 0), stop=(r == GSIZE - 1))
                ot = opool.tile([128, DLOC], f32, name="ot")
                nc.vector.tensor_add(ot[:], acc2[:], po_sb[:, s, :])
                nc.sync.dma_start(out=out_d[ts(s, 128), :], in_=ot[:])

    nc.compile()
    return nc


_CACHE = {}


def _get_compiled(block_plan_key, n_pat):
    key = (block_plan_key, n_pat)
    if key not in _CACHE:
        _CACHE[key] = _build_and_compile(block_plan_key, n_pat)
    return _CACHE[key]


def _plan_from_mask(mask):
    """Derive per-q-chunk k-tile lists + dedup'd 0/1 patterns from the mask.

    Plan entries are (kt, pat_idx or -1, q0): the leading q0 columns of the
    q-chunk are fully masked (skipped in the matmuls); a [KT, KT] 0/1 pattern
    multiplies columns [q0, q0+KT); everything beyond q0+KT must be all-kept.
    """
    keep = mask > -1e20
    if not np.all(mask[keep] == 0.0):
        raise NotImplementedError("only 0/-inf style masks supported")
    pats = []
    pat_index = {}
    plan = []
    for qc in range(NQC):
        qs = slice(qc * QC, (qc + 1) * QC)
        row = []
        for kt in range(NKT):
            ks = slice(kt * KT, (kt + 1) * KT)
            blk = keep[qs, ks]            # [QC, KT]
            if not blk.any():
                continue
            if blk.all():
                row.append((kt, -1))
                continue
            p = np.ascontiguousarray(blk.T).astype(np.float32)  # [KT, QC]
            kb = p.tobytes()
            if kb not in pat_index:
                pat_index[kb] = len(pats)
                pats.append(p)
            row.append((kt, pat_index[kb]))
        plan.append(tuple(row))
    return tuple(plan), pats


def _head_perm():
    """Row permutation per head: even dims first, then odd."""
    perm = []
    for h in range(NH):
        base = h * HD
        perm.extend(base + np.arange(0, HD, 2))
        perm.extend(base + np.arange(1, HD, 2))
    return np.array(perm)


def _pmajor(wT, lo=0, hi=None):
    """[DIM, D] (already transposed weight) -> [128, NIC, hi-lo] partition-
    major layout: out[p, c, d] = wT[c*128 + p, lo + d]."""
    hi = wT.shape[1] if hi is None else hi
    return np.ascontiguousarray(
        wT[:, lo:hi].reshape(NIC, 128, hi - lo).transpose(1, 0, 2)
    ).astype(BF16)


def _prep_in_maps(x, wq, wk, wv, wo, freqs_cos, freqs_sin, pats, n_pat):
    perm = _head_perm()
    wq_p = (wq / math.sqrt(HD))[perm]
    wk_p = wk[perm]

    cosT = np.ascontiguousarray(freqs_cos.T)        # [64, SEQ]
    sinT = np.ascontiguousarray(freqs_sin.T)
    cos2 = np.concatenate([cosT, cosT], axis=0).astype(BF16)   # [128, SEQ]
    sinpm = np.concatenate([-sinT, sinT], axis=0).astype(BF16)

    if n_pat:
        pat_np = np.stack(pats)                     # [n_pat, KT, QC]
    else:
        pat_np = np.zeros((1, KT, QC), dtype=np.float32)
    pat_h = np.ascontiguousarray(pat_np.transpose(1, 0, 2)).astype(BF16)

    # x[b].T -> [128, NQC, NIC, QC]: xh[p, qc, c, q] = xT[c*128+p, qc*QC+q]
    xh = []
    for b in range(BSZ):
        xT = x[b].T.reshape(NIC, 128, NQC, QC)
        xh.append(np.ascontiguousarray(
            xT.transpose(1, 2, 0, 3)).astype(BF16))

    in_maps = []
    for c in range(NCORES):
        b, g = c // GSIZE, c % GSIZE
        rows = slice(g * DLOC, (g + 1) * DLOC)
        wqT = wq_p[rows].T   # [DIM, DLOC]
        wkT = wk_p[rows].T
        in_maps.append({
            "xT": xh[b],
            "wq0T": _pmajor(wqT, 0, HD),
            "wqRT": _pmajor(wqT, HD, DLOC),
            "wk0T": _pmajor(wkT, 0, HD),
            "wkRT": _pmajor(wkT, HD, DLOC),
            "wvT": _pmajor(wv[rows].T),
            "woT": _pmajor(wo[rows].T),
            "cos2": cos2,
            "sinpm": sinpm,
            "pat": pat_h,
        })
    return in_maps


def kernel(x, wq, wk, wv, wo, freqs_cos, freqs_sin, mask):
    x = np.asarray(x, dtype=np.float32)
    wq = np.asarray(wq, dtype=np.float32)
    wk = np.asarray(wk, dtype=np.float32)
    wv = np.asarray(wv, dtype=np.float32)
    wo = np.asarray(wo, dtype=np.float32)
    freqs_cos = np.asarray(freqs_cos, dtype=np.float32)
    freqs_sin = np.asarray(freqs_sin, dtype=np.float32)
    mask = np.asarray(mask, dtype=np.float32)

    plan, pats = _plan_from_mask(mask)
    n_pat = len(pats)
    nc = _get_compiled(plan, n_pat)

    in_maps = _prep_in_maps(x, wq, wk, wv, wo, freqs_cos, freqs_sin,
                            pats, n_pat)

    from concourse.bass_utils import run_bass_kernel_spmd
    res = run_bass_kernel_spmd(nc, in_maps, core_ids=list(range(NCORES)))
    outs = res.results

    full = np.empty((BSZ, SEQ, DIM), dtype=np.float32)
    for c in range(NCORES):
        b, g = c // GSIZE, c % GSIZE
        full[b][:, g * DLOC:(g + 1) * DLOC] = outs[c]["out"]
    return full
